# revision 1
# baseline (speedup 1.0000x reference)
r"""Circulant layer kernel for Trainium2 (8 NeuronCores).

Math: reference computes mv1 + mv2 where
  mv1 = batch_circulant(b) @ d,  mv2 = batch_circulant(d) @ b,
with d = des @ K, b = body @ K.  Both are the circular convolution of d and b
(circular convolution is commutative), so  out = 2 * circconv(d, b).

circconv via DFT:  out = 2 * Re(IDFT(DFT(d) * DFT(b))).
DFT/IDFT are realized as dense matmuls with host-generated constant
cos/sin matrices (input-independent constants).

Sharding: each of the 8 cores owns 128 of the 1024 DFT frequencies.
Per core c:
  KC_c   = K @ CC_c            (1024k x 256s)   fused projection+forward DFT
  DT_c   = KC_c^T @ des^T      (256s x 128b)    \  shares stationary weights
  BT_c   = KC_c^T @ body^T     (256s x 128b)    /
  PT_c   = complex-mult(DT_c, BT_c)             (256s x 128b)  on VectorE
  part_c = (PT_c^T @ G_c)                       (128b x 1024)  inverse DFT
Host sums the 8 partials (unshard).
"""

import numpy as np

import concourse.bass as bass
import concourse.mybir as mybir
import concourse.tile as tile
from concourse.bass_utils import run_bass_kernel_spmd
from concourse.tile_rust import add_dep_helper

B = 128        # batch
D_IN = 1024    # input feature dim (contraction k)
N = 1024       # output feature dim (conv length j) == #frequencies
N_CORES = 8
FPC = N // N_CORES  # frequencies per core (complex)
S = 2 * FPC         # freq slots per core: [0:FPC]=real(cos), [FPC:2FPC]=imag(-sin)

F32 = mybir.dt.float32
F32R = mybir.dt.float32r
BF16 = mybir.dt.bfloat16

# Matmul operand precision: "bf16" (fastest; ~5e-3 rel err), "f32r"
# (single-pass TF32-like; ~3e-4), "f32" (two-pass full fp32; ~7e-7).
import os as _os
MM_PREC = _os.environ.get("CIRC_MM_PREC", "f32r")
MM_DT = {"bf16": BF16, "f32r": F32R, "f32": F32}[MM_PREC]


def _np_in(a):
    """Cast to the matmul precision; bf16 data is shipped packed in fp32
    words (DMA is element-rate-bound: 2-byte elements run at half rate)."""
    import ml_dtypes
    a = np.ascontiguousarray(np.asarray(a, dtype=np.float32))
    if MM_PREC != "bf16":
        return a
    bf = np.ascontiguousarray(a.astype(ml_dtypes.bfloat16))
    return bf.view(np.uint8).reshape(a.shape[0], -1).view(np.float32)

# Number of fp32 transport words per logical input element.
PACK = 2 if MM_PREC == "bf16" else 1
# Transport dtype: bf16 ships packed in fp32 words; f32/f32r ship natively
# (the fp32r verifier requires the producing DMA to be f32r-typed).
TR_DT = F32 if MM_PREC == "bf16" else MM_DT

# Stashed by kernel() for test harnesses that want profiling info.
LAST_RESULT = None

_nc_cache = {}


def _build_nc():
    """Build the (single-program) Bass module run on all 8 cores."""
    nc = bass.Bass(target_bir_lowering=True)

    # Packed inputs: tensors consumed together share one DMA (keeps the
    # per-matmul semaphore-wait count within the ISA limit).
    #   ktcc[j, :D_IN] = K^T,  ktcc[j, D_IN:] = CC   (both indexed by j)
    #   dbt[k, :B] = des^T,    dbt[k, B:] = body^T   (both indexed by k)
    # All inputs are host-packed per SBUF partition: row p holds everything
    # partition p receives, contiguously, so each DMA moves 128 long
    # contiguous rows (DMA throughput is descriptor-rate-bound otherwise).
    XW = (D_IN + S) // PACK
    DW = 2 * B // PACK
    GW = N // PACK
    # ktcc in two contiguous halves, one per HWDGE channel (SP / ACT).
    # Channels are FIFO with ~4-5us latency PER TRANSFER, so one big
    # transfer per channel beats several small ones.
    ktcc_q = [nc.declare_dram_parameter(f"ktcc{i}", [128, 4 * XW], TR_DT, False)
              for i in range(2)]
    # aux = [dbt rows | g rows] packed per partition; goes via gpsimd SWDGE.
    aux = nc.declare_dram_parameter("aux", [128, 8 * DW + 2 * GW], TR_DT, False)  # (s, t) inv DFT rows
    out = nc.declare_dram_parameter("out", [B, N], F32, isOutput=True)
    warm_scratch = nc.dram_tensor("warm_scratch", [1, 4], F32)

    JC = N // 128      # 8 chunks over j (contraction of KC stage)
    KB = D_IN // 128   # 8 blocks over k (output partitions of KC stage)
    SB = S // 128      # 2 blocks over freq slots

    with tile.TileContext(nc) as tc:
        with (
            tc.tile_pool(name="main", bufs=1) as pool,
            tc.tile_pool(name="psum", bufs=1, space="PSUM") as pp,
        ):
            # ---- inputs -> SBUF ----
            # At most 7 input DMAs so the output store lands on the 8th,
            # otherwise-unused HW queue: a DMACopy can carry only ONE sync
            # wait, and the store needs its data-dependency wait — it must
            # not also need a queue-slot wait.
            in_dmas = []
            # All input transfers ride ONE serial SP chain: parallel channels
            # all pay the full ~12us proxy latency, while a serial chain
            # pipelines (first chunk lands ~6us in, rest follow every ~3us).
            ktcc_sb = [pool.tile([128, 4, XW], TR_DT, tag=f"ktcc{q}", name=f"ktcc{q}")
                       for q in range(2)]
            for q in range(2):
                in_dmas.append(nc.sync.dma_start(ktcc_sb[q][:], ktcc_q[q][:, :]))
            aux_raw = pool.tile([128, 8 * DW + 2 * GW], TR_DT, tag="auxr", name="auxr")
            in_dmas.append(nc.sync.dma_start(aux_raw[:], aux[:, :]))
            ktcc_v = [t.bitcast(MM_DT) for t in ktcc_sb]
            kt_sb = [ktcc_v[j // 4][:, j % 4, :D_IN] for j in range(JC)]
            cc_sb = [ktcc_v[j // 4][:, j % 4, D_IN:] for j in range(JC)]
            dbt_sb = pool.tile([128, KB, DW], TR_DT, tag="dbt", name="dbt")
            nc.vector.tensor_copy(dbt_sb[:], aux_raw[:, :8 * DW].rearrange("p (kb w) -> p kb w", kb=KB))
            dbt_v = dbt_sb.bitcast(MM_DT)
            g_stage = pool.tile([128, SB, GW], TR_DT, tag="gst", name="gst")
            nc.vector.tensor_copy(g_stage[:], aux_raw[:, 8 * DW:].rearrange("p (sb w) -> p sb w", sb=SB))
            g_sb = [g_stage.bitcast(MM_DT)[:, s, :] for s in range(SB)]

            # ---- PE warmup: keep the HAM clock un-throttled while DMAs
            # stream in, so the real matmuls all run at 2.4 GHz. Dead-code
            # proofed by a tiny gpsimd DMA of the result to scratch DRAM.
            wz = pool.tile([128, 640], BF16, tag="wz", name="wz")
            nc.gpsimd.memset(wz[:], 0.0)
            wps = pp.tile([128, 512], F32, tag="wps", name="wps")
            for w in range(20):
                nc.tensor.matmul(wps[:], wz[:, :128], wz[:, 128:640],
                                 start=True, stop=True)
            wsb = pool.tile([128, 4], F32, tag="wsb", name="wsb")
            nc.vector.tensor_copy(wsb[:], wps[:, :4])
            warm_dma = nc.gpsimd.dma_start(warm_scratch[:, :], wsb[:1, :])

            # ---- stage 1 + stage 2 interleaved ----
            # KC[k, s] = sum_j KT[j, k] * CC[j, s]; as soon as kc chunk kb is
            # cast to bf16, both stage-2 accumulations consume it, hiding the
            # stage-2 matmuls inside stage-1's DMA-paced gaps.
            kc_sb = [pool.tile([128, S], MM_DT, tag=f"kc{kb}", name=f"kc{kb}") for kb in range(KB)]
            db_all = pool.tile([128, SB, 2 * B], F32, tag="dball", name="dball")
            db_ps = [pp.tile([128, 2 * B], F32, tag=f"dbp{sb}", name=f"dbp{sb}")
                     for sb in range(SB)]
            for kb in range(KB):
                ps = pp.tile([128, S], F32, tag="kcp", name=f"kcp{kb}", bufs=2)
                for j in range(JC):
                    nc.tensor.matmul(
                        ps[:],
                        kt_sb[j][:, kb * 128:(kb + 1) * 128],
                        cc_sb[j][:],
                        start=(j == 0),
                        stop=(j == JC - 1),
                    )
                nc.vector.tensor_copy(kc_sb[kb][:], ps[:])
                if MM_PREC == "bf16":
                    # interleave stage-2 into stage-1's DMA-paced gaps; for
                    # f32/f32r the serialized 4-byte weight loads make this
                    # interleave a net loss, so run stage 2 afterwards.
                    for sb in range(SB):
                        nc.tensor.matmul(db_ps[sb][:],
                                         kc_sb[kb][:, sb * 128:(sb + 1) * 128],
                                         dbt_v[:, kb, :],
                                         start=(kb == 0), stop=(kb == KB - 1))
            if MM_PREC != "bf16":
                for sb in range(SB):
                    for kb in range(KB):
                        nc.tensor.matmul(db_ps[sb][:],
                                         kc_sb[kb][:, sb * 128:(sb + 1) * 128],
                                         dbt_v[:, kb, :],
                                         start=(kb == 0), stop=(kb == KB - 1))
            for sb in range(SB):
                nc.vector.tensor_copy(db_all[:, sb, :], db_ps[sb][:])

            # ---- stage 3: complex pointwise multiply (on freq partitions) ----
            # t01 = [Dr*Br, Dr*Bi], t23 = [Di*Bi, Di*Br]
            # Pr = t01[0] - t23[0],  Pi = t01[1] + t23[1]
            t01 = pool.tile([128, 2, B], F32, tag="t01", name="t01")
            t23 = pool.tile([128, 2, B], F32, tag="t23", name="t23")
            pt = pool.tile([128, 2, B], MM_DT, tag="pt", name="pt")
            dr_b = db_all[:, 0, :B][:, None, :].to_broadcast((128, 2, B))
            di_b = db_all[:, 1, :B][:, None, :].to_broadcast((128, 2, B))
            nc.vector.tensor_mul(t01[:], dr_b, db_all[:, :, B:])
            nc.vector.tensor_mul(t23[:], di_b, db_all[:, ::-1, B:])
            nc.vector.tensor_sub(pt[:, 0, :], t01[:, 0, :], t23[:, 0, :])
            nc.vector.tensor_add(pt[:, 1, :], t01[:, 1, :], t23[:, 1, :])
            pt_sb = [pt[:, sb, :] for sb in range(SB)]

            # ---- stage 4: part = PT^T @ G ----
            out_sb = pool.tile([128, N], F32, tag="outsb", name="outsb")
            last_mm = last_cp = None
            for h in range(2):
                o_ps = pp.tile([128, 512], F32, tag="op", name=f"op{h}", bufs=2)
                for sb in range(SB):
                    last_mm = nc.tensor.matmul(
                        o_ps[:],
                        pt_sb[sb],
                        g_sb[sb][:, h * 512:(h + 1) * 512],
                        start=(sb == 0),
                        stop=(sb == SB - 1),
                    )
                last_cp = nc.vector.tensor_copy(out_sb[:, h * 512:(h + 1) * 512], o_ps[:])
            store_a = nc.sync.dma_start(out[:, :512], out_sb[:, :512])
            store_b = nc.scalar.dma_start(out[:, 512:], out_sb[:, 512:])

            # TileContext's exit emits one tail Drain waiting on every
            # outstanding semaphore; walrus caps instructions at ONE sync
            # wait.  Pre-absorb every tick into SP's clock with a chain of
            # single-wait drains so the tail drain needs none.
            prev = None
            for dep in [*in_dmas, warm_dma, store_a, store_b, last_mm, last_cp]:
                dr = nc.sync.drain(fusable=False)
                add_dep_helper(dr.ins, dep.ins, sync=True,
                               reason="tail: absorb tick into SP clock")
                if prev is not None:
                    add_dep_helper(dr.ins, prev.ins, sync=False,
                                   reason="tail: keep drain chain ordered")
                prev = dr

    return nc


def _dft_constants():
    """Per-core forward (CC) and inverse (G) DFT matrices, float32."""
    j = np.arange(N, dtype=np.float64)
    ccs, gs = [], []
    for c in range(N_CORES):
        f = np.arange(c * FPC, (c + 1) * FPC, dtype=np.float64)
        ang = 2.0 * np.pi * np.outer(j, f) / N          # (j, f)
        cc = np.concatenate([np.cos(ang), -np.sin(ang)], axis=1)   # (N, S)
        # inverse: out[k] = (2/N) * sum_f [Pr cos(2pi f k/N) - Pi sin(2pi f k/N)]
        angT = ang.T                                     # (f, k)
        gr = (2.0 / N) * np.cos(angT)
        gi = -(2.0 / N) * np.sin(angT)
        gmat = np.concatenate([gr, gi], axis=0)          # (S, N)
        ccs.append(np.ascontiguousarray(cc, dtype=np.float32))
        gs.append(np.ascontiguousarray(gmat, dtype=np.float32))
    return ccs, gs


def _partition_pack(a):
    """(R, W) with R = n*128 -> (128, n*W): row p = concat of chunk rows p."""
    r, w = a.shape
    n = r // 128
    return np.ascontiguousarray(
        a.reshape(n, 128, w).transpose(1, 0, 2).reshape(128, n * w))


def kernel(des, body, kernel):
    global LAST_RESULT
    K = np.asarray(kernel, dtype=np.float32)
    kt_np = K.T  # (j, k)
    dbt_np = _partition_pack(_np_in(np.concatenate(
        [np.asarray(des, dtype=np.float32).T, np.asarray(body, dtype=np.float32).T],
        axis=1,
    )))  # (k, 2B) packed
    ccs, gs = _dft_constants()
    ktccs = [
        _partition_pack(_np_in(np.concatenate([kt_np, ccs[c]], axis=1)))
        for c in range(N_CORES)
    ]
    half = ktccs[0].shape[1] // 2
    auxs = [
        np.ascontiguousarray(
            np.concatenate([dbt_np, _partition_pack(_np_in(gs[c]))], axis=1))
        for c in range(N_CORES)
    ]

    if "nc" not in _nc_cache:
        _nc_cache["nc"] = _build_nc()
    nc = _nc_cache["nc"]

    in_maps = [
        {**{f"ktcc{i}": np.ascontiguousarray(ktccs[c][:, i * half:(i + 1) * half])
            for i in range(2)},
         "aux": auxs[c]}
        for c in range(N_CORES)
    ]
    res = run_bass_kernel_spmd(nc, in_maps, list(range(N_CORES)))
    LAST_RESULT = res
    out = np.zeros((B, N), dtype=np.float32)
    for r in res.results:
        out += r["out"]
    return out



# revision 2
# speedup vs baseline: 1.2572x; 1.2572x over previous
r"""Circulant layer kernel for Trainium2 (8 NeuronCores).

Math: reference computes mv1 + mv2 where
  mv1 = batch_circulant(b) @ d,  mv2 = batch_circulant(d) @ b,
with d = des @ K, b = body @ K.  Both are the circular convolution of d and b
(circular convolution is commutative), so  out = 2 * circconv(d, b).

circconv via DFT:  out = 2 * Re(IDFT(DFT(d) * DFT(b))).
DFT/IDFT are realized as dense matmuls with host-generated constant
cos/sin matrices (input-independent constants).

Sharding: each of the 8 cores owns 128 of the 1024 DFT frequencies.
Per core c:
  KC_c   = K @ CC_c            (1024k x 256s)   fused projection+forward DFT
  DT_c   = KC_c^T @ des^T      (256s x 128b)    \  shares stationary weights
  BT_c   = KC_c^T @ body^T     (256s x 128b)    /
  PT_c   = complex-mult(DT_c, BT_c)             (256s x 128b)  on VectorE
  part_c = (PT_c^T @ G_c)                       (128b x 1024)  inverse DFT
Host sums the 8 partials (unshard).
"""

import numpy as np

import concourse.bass as bass
import concourse.mybir as mybir
import concourse.tile as tile
from concourse.bass_utils import run_bass_kernel_spmd
from concourse.tile_rust import add_dep_helper

B = 128        # batch
D_IN = 1024    # input feature dim (contraction k)
N = 1024       # output feature dim (conv length j) == #frequencies
N_CORES = 8
FPC = N // N_CORES  # frequencies per core (complex)
S = 2 * FPC         # freq slots per core: [0:FPC]=real(cos), [FPC:2FPC]=imag(-sin)

F32 = mybir.dt.float32
F32R = mybir.dt.float32r
BF16 = mybir.dt.bfloat16

# Matmul operand precision: "bf16" (fastest; ~5e-3 rel err), "f32r"
# (single-pass TF32-like; ~3e-4), "f32" (two-pass full fp32; ~7e-7).
import os as _os
MM_PREC = _os.environ.get("CIRC_MM_PREC", "bf16")
MM_DT = {"bf16": BF16, "f32r": F32R, "f32": F32}[MM_PREC]


def _np_in(a):
    """Cast to the matmul precision; bf16 data is shipped packed in fp32
    words (DMA is element-rate-bound: 2-byte elements run at half rate)."""
    import ml_dtypes
    a = np.ascontiguousarray(np.asarray(a, dtype=np.float32))
    if MM_PREC != "bf16":
        return a
    bf = np.ascontiguousarray(a.astype(ml_dtypes.bfloat16))
    return bf.view(np.uint8).reshape(a.shape[0], -1).view(np.float32)

# Number of fp32 transport words per logical input element.
PACK = 2 if MM_PREC == "bf16" else 1
# Transport dtype: bf16 ships packed in fp32 words; f32/f32r ship natively
# (the fp32r verifier requires the producing DMA to be f32r-typed).
TR_DT = F32 if MM_PREC == "bf16" else MM_DT

# Stashed by kernel() for test harnesses that want profiling info.
LAST_RESULT = None

_nc_cache = {}


def _build_nc():
    """Build the (single-program) Bass module run on all 8 cores."""
    nc = bass.Bass(target_bir_lowering=True)

    # Packed inputs: tensors consumed together share one DMA (keeps the
    # per-matmul semaphore-wait count within the ISA limit).
    #   ktcc[j, :D_IN] = K^T,  ktcc[j, D_IN:] = CC   (both indexed by j)
    #   dbt[k, :B] = des^T,    dbt[k, B:] = body^T   (both indexed by k)
    # All inputs are host-packed per SBUF partition: row p holds everything
    # partition p receives, contiguously, so each DMA moves 128 long
    # contiguous rows (DMA throughput is descriptor-rate-bound otherwise).
    XW = (D_IN + S) // PACK
    DW = 2 * B // PACK
    GW = N // PACK
    # ktcc in two contiguous halves, one per HWDGE channel (SP / ACT).
    # Channels are FIFO with ~4-5us latency PER TRANSFER, so one big
    # transfer per channel beats several small ones.
    ktcc_q = [nc.declare_dram_parameter(f"ktcc{i}", [128, 4 * XW], TR_DT, False)
              for i in range(2)]
    # aux = [dbt rows | g rows] packed per partition; goes via gpsimd SWDGE.
    aux = nc.declare_dram_parameter("aux", [128, 8 * DW + 2 * GW], TR_DT, False)  # (s, t) inv DFT rows
    out = nc.declare_dram_parameter("out", [B, N], F32, isOutput=True)
    warm_scratch = nc.dram_tensor("warm_scratch", [1, 4], F32)

    JC = N // 128      # 8 chunks over j (contraction of KC stage)
    KB = D_IN // 128   # 8 blocks over k (output partitions of KC stage)
    SB = S // 128      # 2 blocks over freq slots

    with tile.TileContext(nc) as tc:
        with (
            tc.tile_pool(name="main", bufs=1) as pool,
            tc.tile_pool(name="psum", bufs=1, space="PSUM") as pp,
        ):
            # ---- inputs -> SBUF ----
            # At most 7 input DMAs so the output store lands on the 8th,
            # otherwise-unused HW queue: a DMACopy can carry only ONE sync
            # wait, and the store needs its data-dependency wait — it must
            # not also need a queue-slot wait.
            in_dmas = []
            # All input transfers ride ONE serial SP chain: parallel channels
            # all pay the full ~12us proxy latency, while a serial chain
            # pipelines (first chunk lands ~6us in, rest follow every ~3us).
            ktcc_sb = [pool.tile([128, 4, XW], TR_DT, tag=f"ktcc{q}", name=f"ktcc{q}")
                       for q in range(2)]
            for q in range(2):
                in_dmas.append(nc.sync.dma_start(ktcc_sb[q][:], ktcc_q[q][:, :]))
            aux_raw = pool.tile([128, 8 * DW + 2 * GW], TR_DT, tag="auxr", name="auxr")
            in_dmas.append(nc.sync.dma_start(aux_raw[:], aux[:, :]))
            ktcc_v = [t.bitcast(MM_DT) for t in ktcc_sb]
            kt_sb = [ktcc_v[j // 4][:, j % 4, :D_IN] for j in range(JC)]
            cc_sb = [ktcc_v[j // 4][:, j % 4, D_IN:] for j in range(JC)]
            dbt_sb = pool.tile([128, KB, DW], TR_DT, tag="dbt", name="dbt")
            nc.vector.tensor_copy(dbt_sb[:], aux_raw[:, :8 * DW].rearrange("p (kb w) -> p kb w", kb=KB))
            dbt_v = dbt_sb.bitcast(MM_DT)
            g_stage = pool.tile([128, SB, GW], TR_DT, tag="gst", name="gst")
            nc.vector.tensor_copy(g_stage[:], aux_raw[:, 8 * DW:].rearrange("p (sb w) -> p sb w", sb=SB))
            g_sb = [g_stage.bitcast(MM_DT)[:, s, :] for s in range(SB)]

            # ---- PE warmup: keep the HAM clock un-throttled while DMAs
            # stream in, so the real matmuls all run at 2.4 GHz. Dead-code
            # proofed by a tiny gpsimd DMA of the result to scratch DRAM.
            wz = pool.tile([128, 640], BF16, tag="wz", name="wz")
            nc.gpsimd.memset(wz[:], 0.0)
            wps = pp.tile([128, 512], F32, tag="wps", name="wps")
            for w in range(20):
                nc.tensor.matmul(wps[:], wz[:, :128], wz[:, 128:640],
                                 start=True, stop=True)
            wsb = pool.tile([128, 4], F32, tag="wsb", name="wsb")
            nc.vector.tensor_copy(wsb[:], wps[:, :4])
            warm_dma = nc.gpsimd.dma_start(warm_scratch[:, :], wsb[:1, :])

            # ---- stage 1 + stage 2 interleaved ----
            # KC[k, s] = sum_j KT[j, k] * CC[j, s]; as soon as kc chunk kb is
            # cast to bf16, both stage-2 accumulations consume it, hiding the
            # stage-2 matmuls inside stage-1's DMA-paced gaps.
            kc_sb = [pool.tile([128, S], MM_DT, tag=f"kc{kb}", name=f"kc{kb}") for kb in range(KB)]
            db_all = pool.tile([128, SB, 2 * B], F32, tag="dball", name="dball")
            db_ps = [pp.tile([128, 2 * B], F32, tag=f"dbp{sb}", name=f"dbp{sb}")
                     for sb in range(SB)]
            for kb in range(KB):
                ps = pp.tile([128, S], F32, tag="kcp", name=f"kcp{kb}", bufs=2)
                for j in range(JC):
                    nc.tensor.matmul(
                        ps[:],
                        kt_sb[j][:, kb * 128:(kb + 1) * 128],
                        cc_sb[j][:],
                        start=(j == 0),
                        stop=(j == JC - 1),
                    )
                nc.vector.tensor_copy(kc_sb[kb][:], ps[:])
                if MM_PREC == "bf16":
                    # interleave stage-2 into stage-1's DMA-paced gaps; for
                    # f32/f32r the serialized 4-byte weight loads make this
                    # interleave a net loss, so run stage 2 afterwards.
                    for sb in range(SB):
                        nc.tensor.matmul(db_ps[sb][:],
                                         kc_sb[kb][:, sb * 128:(sb + 1) * 128],
                                         dbt_v[:, kb, :],
                                         start=(kb == 0), stop=(kb == KB - 1))
            if MM_PREC != "bf16":
                for sb in range(SB):
                    for kb in range(KB):
                        nc.tensor.matmul(db_ps[sb][:],
                                         kc_sb[kb][:, sb * 128:(sb + 1) * 128],
                                         dbt_v[:, kb, :],
                                         start=(kb == 0), stop=(kb == KB - 1))
            for sb in range(SB):
                nc.vector.tensor_copy(db_all[:, sb, :], db_ps[sb][:])

            # ---- stage 3: complex pointwise multiply (on freq partitions) ----
            # t01 = [Dr*Br, Dr*Bi], t23 = [Di*Bi, Di*Br]
            # Pr = t01[0] - t23[0],  Pi = t01[1] + t23[1]
            t01 = pool.tile([128, 2, B], F32, tag="t01", name="t01")
            t23 = pool.tile([128, 2, B], F32, tag="t23", name="t23")
            pt = pool.tile([128, 2, B], MM_DT, tag="pt", name="pt")
            dr_b = db_all[:, 0, :B][:, None, :].to_broadcast((128, 2, B))
            di_b = db_all[:, 1, :B][:, None, :].to_broadcast((128, 2, B))
            nc.vector.tensor_mul(t01[:], dr_b, db_all[:, :, B:])
            nc.vector.tensor_mul(t23[:], di_b, db_all[:, ::-1, B:])
            nc.vector.tensor_sub(pt[:, 0, :], t01[:, 0, :], t23[:, 0, :])
            nc.vector.tensor_add(pt[:, 1, :], t01[:, 1, :], t23[:, 1, :])
            pt_sb = [pt[:, sb, :] for sb in range(SB)]

            # ---- stage 4: part = PT^T @ G ----
            out_sb = pool.tile([128, N], F32, tag="outsb", name="outsb")
            last_mm = last_cp = None
            for h in range(2):
                o_ps = pp.tile([128, 512], F32, tag="op", name=f"op{h}", bufs=2)
                for sb in range(SB):
                    last_mm = nc.tensor.matmul(
                        o_ps[:],
                        pt_sb[sb],
                        g_sb[sb][:, h * 512:(h + 1) * 512],
                        start=(sb == 0),
                        stop=(sb == SB - 1),
                    )
                last_cp = nc.vector.tensor_copy(out_sb[:, h * 512:(h + 1) * 512], o_ps[:])
            store_a = nc.sync.dma_start(out[:, :512], out_sb[:, :512])
            store_b = nc.scalar.dma_start(out[:, 512:], out_sb[:, 512:])

            # TileContext's exit emits one tail Drain waiting on every
            # outstanding semaphore; walrus caps instructions at ONE sync
            # wait.  Pre-absorb every tick into SP's clock with a chain of
            # single-wait drains so the tail drain needs none.
            prev = None
            for dep in [*in_dmas, warm_dma, store_a, store_b, last_mm, last_cp]:
                dr = nc.sync.drain(fusable=False)
                add_dep_helper(dr.ins, dep.ins, sync=True,
                               reason="tail: absorb tick into SP clock")
                if prev is not None:
                    add_dep_helper(dr.ins, prev.ins, sync=False,
                                   reason="tail: keep drain chain ordered")
                prev = dr

    return nc


def _dft_constants():
    """Per-core forward (CC) and inverse (G) DFT matrices, float32."""
    j = np.arange(N, dtype=np.float64)
    ccs, gs = [], []
    for c in range(N_CORES):
        f = np.arange(c * FPC, (c + 1) * FPC, dtype=np.float64)
        ang = 2.0 * np.pi * np.outer(j, f) / N          # (j, f)
        cc = np.concatenate([np.cos(ang), -np.sin(ang)], axis=1)   # (N, S)
        # inverse: out[k] = (2/N) * sum_f [Pr cos(2pi f k/N) - Pi sin(2pi f k/N)]
        angT = ang.T                                     # (f, k)
        gr = (2.0 / N) * np.cos(angT)
        gi = -(2.0 / N) * np.sin(angT)
        gmat = np.concatenate([gr, gi], axis=0)          # (S, N)
        ccs.append(np.ascontiguousarray(cc, dtype=np.float32))
        gs.append(np.ascontiguousarray(gmat, dtype=np.float32))
    return ccs, gs


def _partition_pack(a):
    """(R, W) with R = n*128 -> (128, n*W): row p = concat of chunk rows p."""
    r, w = a.shape
    n = r // 128
    return np.ascontiguousarray(
        a.reshape(n, 128, w).transpose(1, 0, 2).reshape(128, n * w))


def kernel(des, body, kernel):
    global LAST_RESULT
    K = np.asarray(kernel, dtype=np.float32)
    kt_np = K.T  # (j, k)
    dbt_np = _partition_pack(_np_in(np.concatenate(
        [np.asarray(des, dtype=np.float32).T, np.asarray(body, dtype=np.float32).T],
        axis=1,
    )))  # (k, 2B) packed
    ccs, gs = _dft_constants()
    ktccs = [
        _partition_pack(_np_in(np.concatenate([kt_np, ccs[c]], axis=1)))
        for c in range(N_CORES)
    ]
    half = ktccs[0].shape[1] // 2
    auxs = [
        np.ascontiguousarray(
            np.concatenate([dbt_np, _partition_pack(_np_in(gs[c]))], axis=1))
        for c in range(N_CORES)
    ]

    if "nc" not in _nc_cache:
        _nc_cache["nc"] = _build_nc()
    nc = _nc_cache["nc"]

    in_maps = [
        {**{f"ktcc{i}": np.ascontiguousarray(ktccs[c][:, i * half:(i + 1) * half])
            for i in range(2)},
         "aux": auxs[c]}
        for c in range(N_CORES)
    ]
    res = run_bass_kernel_spmd(nc, in_maps, list(range(N_CORES)))
    LAST_RESULT = res
    out = np.zeros((B, N), dtype=np.float32)
    for r in res.results:
        out += r["out"]
    return out

